# revision 1
# baseline (speedup 1.0000x reference)
"""ComplexGRUCell forward on 8 Trainium2 NeuronCores.

Strategy (data-parallel, feat-major compute):
  - Shard batch B=65536 across 8 cores (8192 rows each).
  - Host-side: transpose x/h slices to [256, 8192] (feature-major) and
    pre-combine the 6 complex weight pairs into 8 stacked real matrices
    (one per pre-activation accumulator), transposed into matmul-stationary
    layout. Biases pre-combined per accumulator.
  - Device: for each 512-column batch tile, accumulate the 8 gate
    pre-activations with fp32r matmuls (features on partitions, batch on the
    free dim), apply sigmoid/tanh with per-partition biases on the scalar
    engine, do the complex-arithmetic elementwise work on the vector engine,
    and DMA the feature-major outputs back.
  - Host-side: transpose outputs back to [B, 256] and stack real/imag.

Self-contained: hardcodes B=65536, I=H=256, 8 cores.
"""

import numpy as np

import concourse.bass as bass  # noqa: F401
import concourse.mybir as mybir
import concourse.tile as tile
from concourse import bacc, bass_utils

F32 = mybir.dt.float32
F32R = mybir.dt.float32r
FP16 = mybir.dt.float16
AF = mybir.ActivationFunctionType

B_TOTAL = 65536
N_CORES = 8
B_LOC = B_TOTAL // N_CORES  # 8192
H = 256
NB = 512                    # batch columns per tile
N_TILES = B_LOC // NB       # 16
KC = H // 128               # 2 feature chunks (partition dim)

_GATE_ACCS = ["r_re", "r_im", "z_re", "z_im"]      # 8 k-chunks each
_CAND_ACCS = ["x3_re", "x3_im", "g3_re", "g3_im"]  # 4 k-chunks each
_STREAMS = ["xrT", "xiT", "hrT", "hiT"]

# Module-level knobs for the test harness (grading path leaves them alone).
TRACE = False
LAST_RESULT = None

_CACHED_NC = None


def _build_nc():
    nc = bacc.Bacc("TRN2", target_bir_lowering=False, debug=False,
                   num_devices=N_CORES)

    ins = {}
    for s in _STREAMS:
        ins[s] = nc.dram_tensor(s, (H, B_LOC), F32R, kind="ExternalInput")
    for g in _GATE_ACCS:
        ins["w_" + g] = nc.dram_tensor("w_" + g, (128, 8 * 256), F32R,
                                       kind="ExternalInput")
    for g in _CAND_ACCS:
        ins["w_" + g] = nc.dram_tensor("w_" + g, (128, 4 * 256), F32R,
                                       kind="ExternalInput")
    ins["biases"] = nc.dram_tensor("biases", (128, 16), F32,
                                   kind="ExternalInput")
    out_r = nc.dram_tensor("outT_r", (H, B_LOC), F32, kind="ExternalOutput")
    out_i = nc.dram_tensor("outT_i", (H, B_LOC), F32, kind="ExternalOutput")

    bias_col = {}
    for gi, g in enumerate(_GATE_ACCS + _CAND_ACCS):
        for mo in range(2):
            bias_col[(g, mo)] = gi * 2 + mo

    with tile.TileContext(nc) as tc:
        with (
            tc.tile_pool(name="wpool", bufs=1) as wpool,
            tc.tile_pool(name="mvpool", bufs=2) as mvpool,
            tc.tile_pool(name="spool", bufs=3) as spool,
            tc.tile_pool(name="tpool", bufs=2) as tpool,
            tc.tile_pool(name="opool", bufs=3) as opool,
            tc.tile_pool(name="psum", bufs=1, space="PSUM") as psum,
        ):
            # ---- one-time weight/bias loads -------------------------------
            # Ordered so only the r-gate weights gate the first matmuls:
            # r weights -> tile-0 data -> remaining weights.
            wt = {}
            wt_chunks = {}

            def load_w(g, n):
                t = wpool.tile([128, n * 256], F32R, name=f"wt_{g}",
                               tag=f"wt_{g}")
                nc.sync.dma_start(t[:], ins["w_" + g][:])
                wt[g] = t

            def load_w_chunked(g, n):
                for ki in range(n):
                    t = wpool.tile([128, 256], F32R, name=f"wt_{g}_{ki}",
                                   tag=f"wt_{g}_{ki}")
                    nc.sync.dma_start(
                        t[:], ins["w_" + g][:, ki * 256:(ki + 1) * 256])
                    wt_chunks[(g, ki)] = t

            def w_ap(g, ki, mo):
                if (g, ki) in wt_chunks:
                    return wt_chunks[(g, ki)][:, mo * 128:(mo + 1) * 128]
                return wt[g][:, ki * 256 + mo * 128:ki * 256 + (mo + 1) * 128]

            def load_mv(c0, nb, streams=(0, 1, 2, 3), mv=None):
                mv = {} if mv is None else mv
                for si in streams:
                    s = _STREAMS[si]
                    for k in range(KC):
                        m = mvpool.tile([128, nb], F32R, name=f"mv{si}{k}",
                                        tag=f"mv{si}{k}",
                                        padded_shape=[128, NB],
                                        bufs=3 if si >= 2 else 2)
                        nc.sync.dma_start(
                            m[:], ins[s][k * 128:(k + 1) * 128, c0:c0 + nb])
                        mv[(si, k)] = m
                return mv

            load_w_chunked("r_re", 8)
            mv0 = load_mv(0, NB, streams=(0, 1))
            load_w_chunked("r_im", 8)
            load_mv(0, NB, streams=(2, 3), mv=mv0)
            for g in ("z_re", "z_im"):
                load_w(g, 8)
            for g in _CAND_ACCS:
                load_w(g, 4)
            bt = wpool.tile([128, 16], F32, name="bias_t", tag="bias_t")
            nc.sync.dma_start(bt[:], ins["biases"][:])

            def bias_ap(g, mo):
                c = bias_col[(g, mo)]
                return bt[:, c:c + 1]

            # ---- per batch tile -------------------------------------------
            schedule = [(i * NB, NB) for i in range(N_TILES)]
            for t_idx, (c0, nb) in enumerate(schedule):
                mv = mv0 if t_idx == 0 else load_mv(c0, nb)

                def mk_pair(nm, tag):
                    return psum.tile([128, 2 * nb], F32, name=nm, tag=tag)

                p_r = [mk_pair(f"p_r{mo}", f"bankA{mo}") for mo in range(2)]
                p_z = [mk_pair(f"p_z{mo}", f"bankB{mo}") for mo in range(2)]

                def accum(pair, half, g, mo, streams):
                    n_mm = len(streams) * KC
                    j = 0
                    for si in streams:
                        for k in range(KC):
                            ki = (si - streams[0]) * KC + k
                            nc.tensor.matmul(
                                pair[:, half * nb:(half + 1) * nb],
                                w_ap(g, ki, mo), mv[(si, k)][:],
                                start=(j == 0), stop=(j == n_mm - 1))
                            j += 1

                ALL, XS, HS = [0, 1, 2, 3], [0, 1], [2, 3]
                for mo in range(2):
                    accum(p_r[mo], 0, "r_re", mo, ALL)
                    accum(p_r[mo], 1, "r_im", mo, ALL)
                    accum(p_z[mo], 0, "z_re", mo, ALL)
                    accum(p_z[mo], 1, "z_im", mo, ALL)

                p_g3 = [mk_pair(f"p_g{mo}", f"bankA{mo}") for mo in range(2)]
                p_x3 = [mk_pair(f"p_x{mo}", f"bankB{mo}") for mo in range(2)]
                for mo in range(2):
                    accum(p_g3[mo], 0, "g3_re", mo, HS)
                    accum(p_g3[mo], 1, "g3_im", mo, HS)
                    accum(p_x3[mo], 0, "x3_re", mo, XS)
                    accum(p_x3[mo], 1, "x3_im", mo, XS)

                # ---- elementwise epilogue per feature chunk ----------------
                for mo in range(2):
                    sr = spool.tile([128, 2 * nb], F32, name=f"sr{mo}", tag="sr", bufs=2)
                    sz = spool.tile([128, 2 * nb], F32, name=f"sz{mo}", tag="sz", bufs=2)
                    g3 = spool.tile([128, 2 * nb], F32, name=f"g3{mo}", tag="g3", bufs=2)
                    nn = spool.tile([128, 2 * nb], F32, name=f"nn{mo}", tag="nn")

                    nc.scalar.activation(sr[:, 0:nb], p_r[mo][:, 0:nb],
                                         AF.Sigmoid, bias=bias_ap("r_re", mo))
                    nc.scalar.activation(sr[:, nb:], p_r[mo][:, nb:],
                                         AF.Sigmoid, bias=bias_ap("r_im", mo))
                    nc.scalar.activation(sz[:, 0:nb], p_z[mo][:, 0:nb],
                                         AF.Sigmoid, bias=bias_ap("z_re", mo))
                    nc.scalar.activation(sz[:, nb:], p_z[mo][:, nb:],
                                         AF.Sigmoid, bias=bias_ap("z_im", mo))
                    nc.scalar.activation(g3[:, 0:nb], p_g3[mo][:, 0:nb],
                                         AF.Identity, bias=bias_ap("g3_re", mo))
                    nc.scalar.activation(g3[:, nb:], p_g3[mo][:, nb:],
                                         AF.Identity, bias=bias_ap("g3_im", mo))

                    # h3 = r * g3 (complex)
                    u = tpool.tile([128, 2 * nb], F32, name=f"u{mo}", tag="u")
                    v = tpool.tile([128, 2 * nb], F32, name=f"v{mo}", tag="v")
                    h3 = tpool.tile([128, 2 * nb], F32, name=f"h3{mo}", tag="h3")
                    ss = tpool.tile([128, 2 * nb], F32, name=f"ss{mo}", tag="ss")
                    nc.vector.tensor_mul(u[:], sr[:], g3[:])   # rr*g3r | ri*g3i
                    nc.vector.tensor_mul(v[:, 0:nb], sr[:, 0:nb], g3[:, nb:])
                    nc.vector.tensor_mul(v[:, nb:], sr[:, nb:], g3[:, 0:nb])
                    nc.vector.tensor_sub(h3[:, 0:nb], u[:, 0:nb], u[:, nb:])
                    nc.vector.tensor_add(h3[:, nb:], v[:, 0:nb], v[:, nb:])
                    # s = x3 + h3 ; n = tanh(s + bias_x3)
                    nc.vector.tensor_add(ss[:], p_x3[mo][:], h3[:])
                    nc.scalar.activation(nn[:, 0:nb], ss[:, 0:nb],
                                         AF.Tanh, bias=bias_ap("x3_re", mo))
                    nc.scalar.activation(nn[:, nb:], ss[:, nb:],
                                         AF.Tanh, bias=bias_ap("x3_im", mo))

                    # d = h - n ; out = n + z*d (complex)
                    d = tpool.tile([128, 2 * nb], F32, name=f"d{mo}", tag="d")
                    p = tpool.tile([128, 2 * nb], F32, name=f"p{mo}", tag="p")
                    q = tpool.tile([128, 2 * nb], F32, name=f"q{mo}", tag="q")
                    tm = tpool.tile([128, 2 * nb], F32, name=f"tm{mo}", tag="tm")
                    ot = opool.tile([128, 2 * nb], F32, name=f"ot{mo}", tag="ot")
                    nc.vector.tensor_sub(d[:, 0:nb],
                                         mv[(2, mo)][:].bitcast(F32), nn[:, 0:nb])
                    nc.vector.tensor_sub(d[:, nb:],
                                         mv[(3, mo)][:].bitcast(F32), nn[:, nb:])
                    nc.vector.tensor_mul(p[:], sz[:], d[:])    # zr*dr | zi*di
                    nc.vector.tensor_mul(q[:, 0:nb], sz[:, 0:nb], d[:, nb:])
                    nc.vector.tensor_mul(q[:, nb:], sz[:, nb:], d[:, 0:nb])
                    nc.vector.tensor_sub(tm[:, 0:nb], p[:, 0:nb], p[:, nb:])
                    nc.vector.tensor_add(tm[:, nb:], q[:, 0:nb], q[:, nb:])
                    nc.vector.tensor_add(ot[:], nn[:], tm[:])

                    nc.sync.dma_start(
                        out_r[mo * 128:(mo + 1) * 128, c0:c0 + nb], ot[:, 0:nb])
                    nc.sync.dma_start(
                        out_i[mo * 128:(mo + 1) * 128, c0:c0 + nb], ot[:, nb:])

    nc.compile()
    return nc


def _prep_weights(p):
    """Host-side weight/bias combination -> device layouts."""
    def stk(mats):  # list of [256,256] -> stationary layout [128, n*256]
        W = np.concatenate(mats, axis=1)          # [out=256, in_total]
        WT = np.ascontiguousarray(W.T)            # [in_total, 256]
        n = WT.shape[0] // 128
        return np.ascontiguousarray(
            WT.reshape(n, 128, 256).transpose(1, 0, 2).reshape(128, n * 256)
        ).astype(np.float32)

    w = {}
    w["w_r_re"] = stk([p["w1Wr"], -p["w1Wi"], p["r1Wr"], -p["r1Wi"]])
    w["w_r_im"] = stk([p["w1Wi"], p["w1Wr"], p["r1Wi"], p["r1Wr"]])
    w["w_z_re"] = stk([p["w2Wr"], -p["w2Wi"], p["r2Wr"], -p["r2Wi"]])
    w["w_z_im"] = stk([p["w2Wi"], p["w2Wr"], p["r2Wi"], p["r2Wr"]])
    w["w_x3_re"] = stk([p["w3Wr"], -p["w3Wi"]])
    w["w_x3_im"] = stk([p["w3Wi"], p["w3Wr"]])
    w["w_g3_re"] = stk([p["r3Wr"], -p["r3Wi"]])
    w["w_g3_im"] = stk([p["r3Wi"], p["r3Wr"]])

    bias = {
        "r_re": p["w1br"] - p["w1bi"] + p["r1br"] - p["r1bi"],
        "r_im": p["w1br"] + p["w1bi"] + p["r1br"] + p["r1bi"],
        "z_re": p["w2br"] - p["w2bi"] + p["r2br"] - p["r2bi"],
        "z_im": p["w2br"] + p["w2bi"] + p["r2br"] + p["r2bi"],
        "x3_re": p["w3br"] - p["w3bi"],
        "x3_im": p["w3br"] + p["w3bi"],
        "g3_re": p["r3br"] - p["r3bi"],
        "g3_im": p["r3br"] + p["r3bi"],
    }
    bcols = np.zeros((128, 16), dtype=np.float32)
    for gi, g in enumerate(_GATE_ACCS + _CAND_ACCS):
        for mo in range(2):
            bcols[:, gi * 2 + mo] = np.asarray(bias[g])[mo * 128:(mo + 1) * 128]
    w["biases"] = bcols
    return w


def kernel(**inputs):
    global _CACHED_NC, LAST_RESULT
    if _CACHED_NC is None:
        _CACHED_NC = _build_nc()
    nc = _CACHED_NC

    wmaps = _prep_weights(inputs)

    in_maps = []
    for c in range(N_CORES):
        sl = slice(c * B_LOC, (c + 1) * B_LOC)
        m = dict(wmaps)
        m["xrT"] = np.ascontiguousarray(np.asarray(inputs["xr"])[sl].T,
                                        dtype=np.float32)
        m["xiT"] = np.ascontiguousarray(np.asarray(inputs["xi"])[sl].T,
                                        dtype=np.float32)
        m["hrT"] = np.ascontiguousarray(np.asarray(inputs["hr"])[sl].T,
                                        dtype=np.float32)
        m["hiT"] = np.ascontiguousarray(np.asarray(inputs["hi"])[sl].T,
                                        dtype=np.float32)
        in_maps.append(m)

    kwargs = {}
    if TRACE:
        import sys, types
        try:
            from trn_agent_boot.trn_boot import _ntff_profile_via_ctypes
            mod = types.ModuleType("antenv.axon_hooks")
            mod._hook = _ntff_profile_via_ctypes('/opt/axon/libaxon_pjrt.so')
            mod.get_axon_ntff_profile_hook = lambda: mod._hook
            mod.set_axon_ntff_profile_hook = (
                lambda h: setattr(mod, "_hook", h))
            sys.modules["antenv.axon_hooks"] = mod
            kwargs["trace"] = True
        except Exception:
            pass

    res = bass_utils.run_bass_kernel_spmd(
        nc, in_maps, core_ids=list(range(N_CORES)), **kwargs)
    LAST_RESULT = res

    out = np.empty((2, B_TOTAL, H), dtype=np.float32)
    for c in range(N_CORES):
        sl = slice(c * B_LOC, (c + 1) * B_LOC)
        out[0, sl] = res.results[c]["outT_r"].T
        out[1, sl] = res.results[c]["outT_i"].T
    return out



# revision 5
# speedup vs baseline: 1.0408x; 1.0408x over previous
"""ComplexGRUCell forward on 8 Trainium2 NeuronCores.

Strategy (data-parallel, feat-major compute), v3:
  - Shard batch B=65536 across 8 cores (8192 rows each).
  - Gate pre-activations (r, z) via fp8-e4m3 DoubleRowSwInterleave
    matmuls (2x PE throughput, software-interleaved weights so the
    weight loads read contiguously). The moving data is split hi+lo
    (error feedback): two DR matmuls per weight chunk accumulate
    w*(x_hi) + w*(x_lo), cancelling the moving-side quantization error.
    Host scales moving data by 16 and weights by 256; the sigmoid
    un-scales via its scale argument.
  - Candidate pre-activations (x3, g3) via fp16 matmuls.
  - All element-wise epilogue work in fp16 SBUF (2x packed DVE mode).
  - Streams shipped feature-major, tile-major interleaved so each batch
    tile needs ONE dma per stream class (fp16 / fp8-hi / fp8-lo).
  - Outputs written fp16 [256, 2, 8192]; host transposes/upcasts.

Self-contained: hardcodes B=65536, I=H=256, 8 cores.
"""

import numpy as np
import ml_dtypes

import concourse.bass as bass  # noqa: F401
import concourse.mybir as mybir
import concourse.tile as tile
from concourse import bacc, bass_utils

F32 = mybir.dt.float32
F16 = mybir.dt.float16
F8 = mybir.dt.float8e4
AF = mybir.ActivationFunctionType
PM = mybir.MatmulPerfMode

B_TOTAL = 65536
N_CORES = 8
B_LOC = B_TOTAL // N_CORES  # 8192
H = 256
NB = 512                    # batch columns per tile
N_TILES = B_LOC // NB       # 16

S_MOV = 16.0                # fp8 moving-data scale
S_WGT = 256.0               # fp8 weight scale
INV_S = 1.0 / (S_MOV * S_WGT)
E4M3 = ml_dtypes.float8_e4m3

GATE_PERF = PM.DoubleRowSwInterleave   # or PM.DoubleRow
SPLIT_MOV = True                        # hi+lo error feedback on moving data

_STREAMS = ["xr", "xi", "hr", "hi"]
_GATE_ACCS = ["r_re", "r_im", "z_re", "z_im"]
_CAND_ACCS = ["x3_re", "x3_im", "g3_re", "g3_im"]

# Module-level knobs for the test harness (grading path leaves them alone).
TRACE = False
LAST_RESULT = None

_CACHED_NC = None


def _build_nc():
    nc = bacc.Bacc("TRN2", target_bir_lowering=False, debug=False,
                   num_devices=N_CORES)

    ins = {}
    # tile-major interleaved streams: [128, t, 2*si + c, b]
    ins["s16"] = nc.dram_tensor("s16", (128, N_TILES * 8 * NB), F16,
                                kind="ExternalInput")
    ins["s8h"] = nc.dram_tensor("s8h", (128, N_TILES * 8 * NB), F8,
                                kind="ExternalInput")
    if SPLIT_MOV:
        ins["s8l"] = nc.dram_tensor("s8l", (128, N_TILES * 8 * NB), F8,
                                    kind="ExternalInput")
    # gate weights (fp8, SwInterleave or DR pair layout): per (acc,mo):
    # 4 si-blocks of 256 cols
    ins["w8"] = nc.dram_tensor("w8", (128, 8, 4 * 256), F8,
                               kind="ExternalInput")
    # cand weights fp16: per (acc,mo): 4 chunks of 128 cols
    ins["w16"] = nc.dram_tensor("w16", (128, 8, 4 * 128), F16,
                                kind="ExternalInput")
    ins["biases"] = nc.dram_tensor("biases", (128, 16), F32,
                                   kind="ExternalInput")
    # output: [feature, re/im, batch]
    outT = nc.dram_tensor("outT", (H, 2, B_LOC), F16, kind="ExternalOutput")

    gate_idx = {g: i for i, g in enumerate(_GATE_ACCS)}
    cand_idx = {g: i for i, g in enumerate(_CAND_ACCS)}
    bias_col = {}
    for gi, g in enumerate(_GATE_ACCS + _CAND_ACCS):
        for mo in range(2):
            bias_col[(g, mo)] = gi * 2 + mo

    with tile.TileContext(nc) as tc:
        with (
            tc.tile_pool(name="wpool", bufs=1) as wpool,
            tc.tile_pool(name="m8pool", bufs=2) as m8pool,
            tc.tile_pool(name="m16pool", bufs=3) as m16pool,
            tc.tile_pool(name="spool", bufs=2) as spool,
            tc.tile_pool(name="tpool", bufs=2) as tpool,
            tc.tile_pool(name="opool", bufs=3) as opool,
            tc.tile_pool(name="psum", bufs=1, space="PSUM") as psum,
        ):
            # ---- one-time weight/bias loads -------------------------------
            w8t = wpool.tile([128, 8, 4 * 256], F8, name="w8t", tag="w8t")
            nc.sync.dma_start(w8t[:], ins["w8"][:])

            def load_m8(c0):
                t0 = c0 // NB * (8 * NB)
                h8 = m8pool.tile([128, 8, NB], F8, name="m8h", tag="m8h")
                nc.sync.dma_start(h8[:], ins["s8h"][:, t0:t0 + 8 * NB])
                l8 = None
                if SPLIT_MOV:
                    l8 = m8pool.tile([128, 8, NB], F8, name="m8l", tag="m8l")
                    nc.sync.dma_start(l8[:], ins["s8l"][:, t0:t0 + 8 * NB])
                return h8, l8

            def load_m16(c0):
                t0 = c0 // NB * (8 * NB)
                t = m16pool.tile([128, 8, NB], F16, name="m16", tag="m16")
                nc.sync.dma_start(t[:], ins["s16"][:, t0:t0 + 8 * NB])
                return t

            m8_0 = load_m8(0)
            w16t = wpool.tile([128, 8, 4 * 128], F16, name="w16t", tag="w16t")
            nc.sync.dma_start(w16t[:], ins["w16"][:])
            m16_0 = load_m16(0)
            bt = wpool.tile([128, 16], F32, name="bias_t", tag="bias_t")
            nc.sync.dma_start(bt[:], ins["biases"][:])

            def bias_ap(g, mo):
                c = bias_col[(g, mo)]
                return bt[:, c:c + 1]

            # ---- per batch tile -------------------------------------------
            for t_idx in range(N_TILES):
                c0 = t_idx * NB
                if t_idx == 0:
                    (m8h, m8l), m16 = m8_0, m16_0
                else:
                    m8h, m8l = load_m8(c0)
                    m16 = load_m16(c0)

                for mo in range(2):
                    p_r = psum.tile([128, 2 * NB], F32, name=f"pr{mo}",
                                    tag="bkA")
                    p_z = psum.tile([128, 2 * NB], F32, name=f"pz{mo}",
                                    tag="bkB")

                    def gate_accum(dst, g, mo):
                        wrow = gate_idx[g] * 2 + mo
                        n_mm = 8 if SPLIT_MOV else 4
                        j = 0
                        for si in range(4):
                            w = w8t[:, wrow, si * 256:(si + 1) * 256]
                            movs = ([m8h, m8l] if SPLIT_MOV else [m8h])
                            for mv in movs:
                                nc.tensor.matmul(
                                    dst, w, mv[:, 2 * si:2 * si + 2, :],
                                    start=(j == 0), stop=(j == n_mm - 1),
                                    perf_mode=GATE_PERF)
                                j += 1

                    gate_accum(p_r[:, 0:NB], "r_re", mo)
                    gate_accum(p_r[:, NB:], "r_im", mo)
                    gate_accum(p_z[:, 0:NB], "z_re", mo)
                    gate_accum(p_z[:, NB:], "z_im", mo)

                    p_x3 = psum.tile([128, 2 * NB], F32, name=f"px{mo}",
                                     tag="bkC")
                    p_g3 = psum.tile([128, 2 * NB], F32, name=f"pg{mo}",
                                     tag="bkD")

                    def cand_accum(dst, g, mo, j0):
                        wrow = cand_idx[g] * 2 + mo
                        for k in range(4):
                            nc.tensor.matmul(
                                dst, w16t[:, wrow, k * 128:(k + 1) * 128],
                                m16[:, j0 + k, :], start=(k == 0),
                                stop=(k == 3))

                    cand_accum(p_x3[:, 0:NB], "x3_re", mo, 0)
                    cand_accum(p_x3[:, NB:], "x3_im", mo, 0)
                    cand_accum(p_g3[:, 0:NB], "g3_re", mo, 4)
                    cand_accum(p_g3[:, NB:], "g3_im", mo, 4)

                    # ---- elementwise epilogue ------------------------------
                    sr = spool.tile([128, 2 * NB], F16, name=f"sr{mo}",
                                    tag="sr")
                    sz = spool.tile([128, 2 * NB], F16, name=f"sz{mo}",
                                    tag="sz")
                    g3 = spool.tile([128, 2 * NB], F16, name=f"g3{mo}",
                                    tag="g3")
                    nc.scalar.activation(sr[:, 0:NB], p_r[:, 0:NB],
                                         AF.Sigmoid, bias=bias_ap("r_re", mo),
                                         scale=INV_S)
                    nc.scalar.activation(sr[:, NB:], p_r[:, NB:],
                                         AF.Sigmoid, bias=bias_ap("r_im", mo),
                                         scale=INV_S)
                    nc.scalar.activation(sz[:, 0:NB], p_z[:, 0:NB],
                                         AF.Sigmoid, bias=bias_ap("z_re", mo),
                                         scale=INV_S)
                    nc.scalar.activation(sz[:, NB:], p_z[:, NB:],
                                         AF.Sigmoid, bias=bias_ap("z_im", mo),
                                         scale=INV_S)
                    nc.scalar.activation(g3[:, 0:NB], p_g3[:, 0:NB],
                                         AF.Identity,
                                         bias=bias_ap("g3_re", mo))
                    nc.scalar.activation(g3[:, NB:], p_g3[:, NB:],
                                         AF.Identity,
                                         bias=bias_ap("g3_im", mo))

                    # h3 = r * g3 (complex), all fp16 SBUF (2x DVE mode)
                    u = tpool.tile([128, 2 * NB], F16, name=f"u{mo}", tag="u")
                    v = tpool.tile([128, 2 * NB], F16, name=f"v{mo}", tag="v")
                    h3 = tpool.tile([128, 2 * NB], F16, name=f"h3{mo}",
                                    tag="h3")
                    nc.vector.tensor_mul(u[:], sr[:], g3[:])
                    nc.vector.tensor_mul(v[:, 0:NB], sr[:, 0:NB], g3[:, NB:])
                    nc.vector.tensor_mul(v[:, NB:], sr[:, NB:], g3[:, 0:NB])
                    nc.vector.tensor_sub(h3[:, 0:NB], u[:, 0:NB], u[:, NB:])
                    nc.vector.tensor_add(h3[:, NB:], v[:, 0:NB], v[:, NB:])
                    # ss = x3 + h3 (PSUM read, 1x); tanh adds x3 bias
                    ss = tpool.tile([128, 2 * NB], F16, name=f"ss{mo}",
                                    tag="ss")
                    nc.vector.tensor_add(ss[:], p_x3[:], h3[:])
                    nn = spool.tile([128, 2 * NB], F16, name=f"nn{mo}",
                                    tag="nn")
                    nc.scalar.activation(nn[:, 0:NB], ss[:, 0:NB], AF.Tanh,
                                         bias=bias_ap("x3_re", mo))
                    nc.scalar.activation(nn[:, NB:], ss[:, NB:], AF.Tanh,
                                         bias=bias_ap("x3_im", mo))

                    # d = h - n ; out = n + z*d (complex)
                    d = tpool.tile([128, 2 * NB], F16, name=f"d{mo}", tag="d")
                    p = tpool.tile([128, 2 * NB], F16, name=f"p{mo}", tag="p")
                    q = tpool.tile([128, 2 * NB], F16, name=f"q{mo}", tag="q")
                    tm = tpool.tile([128, 2 * NB], F16, name=f"tm{mo}",
                                    tag="tm")
                    ot = opool.tile([128, 2, NB], F16, name=f"ot{mo}",
                                    tag="ot")
                    nc.vector.tensor_sub(d[:, 0:NB], m16[:, 4 + mo, :],
                                         nn[:, 0:NB])
                    nc.vector.tensor_sub(d[:, NB:], m16[:, 6 + mo, :],
                                         nn[:, NB:])
                    nc.vector.tensor_mul(p[:], sz[:], d[:])
                    nc.vector.tensor_mul(q[:, 0:NB], sz[:, 0:NB], d[:, NB:])
                    nc.vector.tensor_mul(q[:, NB:], sz[:, NB:], d[:, 0:NB])
                    nc.vector.tensor_sub(tm[:, 0:NB], p[:, 0:NB], p[:, NB:])
                    nc.vector.tensor_add(tm[:, NB:], q[:, 0:NB], q[:, NB:])
                    nc.vector.tensor_add(ot[:, 0, :], nn[:, 0:NB],
                                         tm[:, 0:NB])
                    nc.vector.tensor_add(ot[:, 1, :], nn[:, NB:], tm[:, NB:])

                    # one DMA per mo: [128 feat, 2 (re/im), NB]
                    nc.sync.dma_start(
                        outT[mo * 128:(mo + 1) * 128, :, c0:c0 + NB], ot[:])

    nc.compile()
    return nc


def _stack_stat(p, g):
    """Stationary matrix [K, 256] for accumulator g (K = 1024 or 512)."""
    blocks = {
        "r_re": [p["w1Wr"], -p["w1Wi"], p["r1Wr"], -p["r1Wi"]],
        "r_im": [p["w1Wi"], p["w1Wr"], p["r1Wi"], p["r1Wr"]],
        "z_re": [p["w2Wr"], -p["w2Wi"], p["r2Wr"], -p["r2Wi"]],
        "z_im": [p["w2Wi"], p["w2Wr"], p["r2Wi"], p["r2Wr"]],
        "x3_re": [p["w3Wr"], -p["w3Wi"]],
        "x3_im": [p["w3Wi"], p["w3Wr"]],
        "g3_re": [p["r3Wr"], -p["r3Wi"]],
        "g3_im": [p["r3Wi"], p["r3Wr"]],
    }[g]
    return np.concatenate([np.asarray(W, np.float32).T for W in blocks],
                          axis=0)


def _pack_gate_pair(w0, w1):
    """Pack a chunk pair [128,128]x2 into the DR weight layout [128, 256]."""
    if GATE_PERF == PM.DoubleRowSwInterleave:
        # flat[p, 2*(127-m) + i] = w_i[p, m]
        arr = np.stack([w0[:, ::-1], w1[:, ::-1]], axis=2)  # [p, m', i]
        return arr.reshape(128, 256)
    # plain DoubleRow: [p, i, m]
    return np.stack([w0, w1], axis=1).reshape(128, 256)


def _prep_weights(p):
    w8 = np.zeros((128, 8, 4 * 256), dtype=np.float32)
    for g in _GATE_ACCS:
        stat = _stack_stat(p, g)  # [1024, 256]
        for mo in range(2):
            sub = stat[:, mo * 128:(mo + 1) * 128] * S_WGT  # [1024, 128]
            gi = _GATE_ACCS.index(g)
            for si in range(4):
                w0 = sub[si * 256:si * 256 + 128]
                w1 = sub[si * 256 + 128:(si + 1) * 256]
                w8[:, gi * 2 + mo, si * 256:(si + 1) * 256] = \
                    _pack_gate_pair(w0, w1)
    w16 = np.zeros((128, 8, 4 * 128), dtype=np.float16)
    for g in _CAND_ACCS:
        stat = _stack_stat(p, g)  # [512, 256]
        for mo in range(2):
            sub = stat[:, mo * 128:(mo + 1) * 128]
            gi = _CAND_ACCS.index(g)
            for k in range(4):
                w16[:, gi * 2 + mo, k * 128:(k + 1) * 128] = \
                    sub[k * 128:(k + 1) * 128].astype(np.float16)

    bias = {
        "r_re": p["w1br"] - p["w1bi"] + p["r1br"] - p["r1bi"],
        "r_im": p["w1br"] + p["w1bi"] + p["r1br"] + p["r1bi"],
        "z_re": p["w2br"] - p["w2bi"] + p["r2br"] - p["r2bi"],
        "z_im": p["w2br"] + p["w2bi"] + p["r2br"] + p["r2bi"],
        "x3_re": p["w3br"] - p["w3bi"],
        "x3_im": p["w3br"] + p["w3bi"],
        "g3_re": p["r3br"] - p["r3bi"],
        "g3_im": p["r3br"] + p["r3bi"],
    }
    bcols = np.zeros((128, 16), dtype=np.float32)
    for gi, g in enumerate(_GATE_ACCS + _CAND_ACCS):
        for mo in range(2):
            bcols[:, gi * 2 + mo] = np.asarray(bias[g])[mo * 128:(mo + 1) * 128]
    return {
        "w8": np.clip(w8, -240.0, 240.0).astype(E4M3),
        "w16": w16,
        "biases": bcols,
    }


def kernel(**inputs):
    global _CACHED_NC, LAST_RESULT
    if _CACHED_NC is None:
        _CACHED_NC = _build_nc()
    nc = _CACHED_NC

    wmaps = _prep_weights(inputs)

    in_maps = []
    for core in range(N_CORES):
        sl = slice(core * B_LOC, (core + 1) * B_LOC)
        m = dict(wmaps)
        # [4 streams][256 feat, 8192 batch]
        xT = np.stack([
            np.asarray(inputs[s], np.float32)[sl].T for s in _STREAMS])
        # -> [128, t, 2*si + c, b] : chunks c over feature dim
        xTi = (xT.reshape(4, 2, 128, N_TILES, NB)
               .transpose(2, 3, 0, 1, 4))          # [128, t, si, c, b]
        m["s16"] = np.ascontiguousarray(
            xTi.reshape(128, -1).astype(np.float16))
        sc = np.clip(xTi * S_MOV, -240.0, 240.0)
        hi = sc.astype(E4M3)
        m["s8h"] = np.ascontiguousarray(hi.reshape(128, -1))
        if SPLIT_MOV:
            lo = (sc - hi.astype(np.float32)).astype(E4M3)
            m["s8l"] = np.ascontiguousarray(lo.reshape(128, -1))
        in_maps.append(m)

    kwargs = {}
    if TRACE:
        import sys, types
        try:
            from trn_agent_boot.trn_boot import _ntff_profile_via_ctypes
            mod = types.ModuleType("antenv.axon_hooks")
            mod._hook = _ntff_profile_via_ctypes('/opt/axon/libaxon_pjrt.so')
            mod.get_axon_ntff_profile_hook = lambda: mod._hook
            mod.set_axon_ntff_profile_hook = (
                lambda h: setattr(mod, "_hook", h))
            sys.modules["antenv.axon_hooks"] = mod
            kwargs["trace"] = True
        except Exception:
            pass

    res = bass_utils.run_bass_kernel_spmd(
        nc, in_maps, core_ids=list(range(N_CORES)), **kwargs)
    LAST_RESULT = res

    out = np.empty((2, B_TOTAL, H), dtype=np.float32)
    for core in range(N_CORES):
        sl = slice(core * B_LOC, (core + 1) * B_LOC)
        o = np.asarray(res.results[core]["outT"], dtype=np.float32)
        out[0, sl] = o[:, 0, :].T
        out[1, sl] = o[:, 1, :].T
    return out


# revision 16
# speedup vs baseline: 1.3154x; 1.2639x over previous
"""ComplexGRUCell forward on 8 Trainium2 NeuronCores.

Strategy (data-parallel, feat-major compute), v3:
  - Shard batch B=65536 across 8 cores (8192 rows each).
  - Gate pre-activations (r, z) via fp8-e4m3 DoubleRowSwInterleave
    matmuls (2x PE throughput, software-interleaved weights so the
    weight loads read contiguously). The moving data is split hi+lo
    (error feedback): two DR matmuls per weight chunk accumulate
    w*(x_hi) + w*(x_lo), cancelling the moving-side quantization error.
    Host scales moving data by 16 and weights by 256; the sigmoid
    un-scales via its scale argument.
  - Candidate pre-activations (x3, g3) via fp16 matmuls.
  - All element-wise epilogue work in fp16 SBUF (2x packed DVE mode).
  - Streams shipped feature-major, tile-major interleaved so each batch
    tile needs ONE dma per stream class (fp16 / fp8-hi / fp8-lo).
  - Outputs written fp16 [256, 2, 8192]; host transposes/upcasts.

Self-contained: hardcodes B=65536, I=H=256, 8 cores.
"""

import numpy as np
import ml_dtypes

import concourse.bass as bass  # noqa: F401
import concourse.mybir as mybir
import concourse.tile as tile
from concourse import bacc, bass_utils

F32 = mybir.dt.float32
F16 = mybir.dt.float16
F8 = mybir.dt.float8e4
AF = mybir.ActivationFunctionType
PM = mybir.MatmulPerfMode

B_TOTAL = 65536
N_CORES = 8
B_LOC = B_TOTAL // N_CORES  # 8192
H = 256
NB = 512                    # batch columns per tile
N_TILES = B_LOC // NB       # 16

S_MOV = 16.0                # fp8 moving-data scale
S_WGT = 256.0               # fp8 weight scale
INV_S = 1.0 / (S_MOV * S_WGT)
E4M3 = ml_dtypes.float8_e4m3

GATE_PERF = PM.DoubleRow

_STREAMS = ["xr", "xi", "hr", "hi"]
_R_ACCS = ["r_re", "r_im"]                       # fp8 DoubleRow
_Z_ACCS = ["z_re", "z_im"]                       # fp16 (precision-critical)
_GATE_ACCS = _R_ACCS + _Z_ACCS
_CAND_ACCS = ["x3_re", "x3_im", "g3_re", "g3_im"]

# Module-level knobs for the test harness (grading path leaves them alone).
TRACE = False
LAST_RESULT = None

_CACHED_NC = None


def _build_nc():
    nc = bacc.Bacc("TRN2", target_bir_lowering=False, debug=False,
                   num_devices=N_CORES)

    ins = {}
    # tile-major interleaved streams: [128, t, 2*si + c, b]
    ins["s16"] = nc.dram_tensor("s16", (128, N_TILES * 8 * NB), F16,
                                kind="ExternalInput")
    ins["s8h"] = nc.dram_tensor("s8h", (128, N_TILES * 8 * NB), F8,
                                kind="ExternalInput")
    # r-gate weights (fp8, DR pair layout): per (acc,mo): 4 si-blocks
    ins["w8"] = nc.dram_tensor("w8", (128, 4, 4 * 256), F8,
                               kind="ExternalInput")
    # z-gate weights fp16: per (acc,mo): 8 chunks of 128 cols
    ins["w16z"] = nc.dram_tensor("w16z", (128, 4, 8 * 128), F16,
                                 kind="ExternalInput")
    # cand weights fp16: per (acc,mo): 4 chunks of 128 cols
    ins["w16"] = nc.dram_tensor("w16", (128, 8, 4 * 128), F16,
                                kind="ExternalInput")
    ins["biases"] = nc.dram_tensor("biases", (128, 16), F32,
                                   kind="ExternalInput")
    # output: [feature, re/im, batch]
    outT = nc.dram_tensor("outT", (H, 2, B_LOC), F16, kind="ExternalOutput")

    r_idx = {g: i for i, g in enumerate(_R_ACCS)}
    z_idx = {g: i for i, g in enumerate(_Z_ACCS)}
    cand_idx = {g: i for i, g in enumerate(_CAND_ACCS)}
    bias_col = {}
    for gi, g in enumerate(_GATE_ACCS + _CAND_ACCS):
        for mo in range(2):
            bias_col[(g, mo)] = gi * 2 + mo

    with tile.TileContext(nc) as tc:
        with (
            tc.tile_pool(name="wpool", bufs=1) as wpool,
            tc.tile_pool(name="m8pool", bufs=2) as m8pool,
            tc.tile_pool(name="m16pool", bufs=3) as m16pool,
            tc.tile_pool(name="spool", bufs=2) as spool,
            tc.tile_pool(name="tpool", bufs=2) as tpool,
            tc.tile_pool(name="opool", bufs=3) as opool,
            tc.tile_pool(name="psum", bufs=1, space="PSUM") as psum,
        ):
            # ---- one-time weight/bias loads -------------------------------
            # [128, (row*4+si)*2 + j, 128]: DR pair dim must be its own axis
            w8t = wpool.tile([128, 32, 128], F8, name="w8t", tag="w8t")
            nc.sync.dma_start(w8t[:], ins["w8"][:])

            def load_m8(c0):
                t0 = c0 // NB * (8 * NB)
                h8 = m8pool.tile([128, 8, NB], F8, name="m8h", tag="m8h")
                nc.sync.dma_start(h8[:], ins["s8h"][:, t0:t0 + 8 * NB])
                return h8

            def load_m16(c0):
                t0 = c0 // NB * (8 * NB)
                t = m16pool.tile([128, 8, NB], F16, name="m16", tag="m16")
                nc.sync.dma_start(t[:], ins["s16"][:, t0:t0 + 8 * NB])
                return t

            m8_0 = load_m8(0)
            wzt = wpool.tile([128, 4, 8 * 128], F16, name="wzt", tag="wzt")
            nc.sync.dma_start(wzt[:], ins["w16z"][:])
            m16_0 = load_m16(0)
            w16t = wpool.tile([128, 8, 4 * 128], F16, name="w16t", tag="w16t")
            nc.sync.dma_start(w16t[:], ins["w16"][:])
            bt = wpool.tile([128, 16], F32, name="bias_t", tag="bias_t")
            nc.sync.dma_start(bt[:], ins["biases"][:])

            def bias_ap(g, mo):
                c = bias_col[(g, mo)]
                return bt[:, c:c + 1]

            # ---- per batch tile -------------------------------------------
            for t_idx in range(N_TILES):
                c0 = t_idx * NB
                if t_idx == 0:
                    m8h, m16 = m8_0, m16_0
                else:
                    m8h = load_m8(c0)
                    m16 = load_m16(c0)

                for mo in range(2):
                    p_r = psum.tile([128, 2 * NB], F32, name=f"pr{mo}",
                                    tag="bkA")
                    p_z = psum.tile([128, 2 * NB], F32, name=f"pz{mo}",
                                    tag="bkB")

                    def r_accum(dst, g, mo):
                        wrow = r_idx[g] * 2 + mo
                        for si in range(4):
                            k = (wrow * 4 + si) * 2
                            nc.tensor.matmul(
                                dst, w8t[:, k:k + 2, :],
                                m8h[:, 2 * si:2 * si + 2, :],
                                start=(si == 0), stop=(si == 3),
                                perf_mode=GATE_PERF)

                    def z_accum(dst, g, mo):
                        wrow = z_idx[g] * 2 + mo
                        for j in range(8):
                            nc.tensor.matmul(
                                dst, wzt[:, wrow, j * 128:(j + 1) * 128],
                                m16[:, j, :], start=(j == 0), stop=(j == 7))

                    r_accum(p_r[:, 0:NB], "r_re", mo)
                    r_accum(p_r[:, NB:], "r_im", mo)
                    z_accum(p_z[:, 0:NB], "z_re", mo)
                    z_accum(p_z[:, NB:], "z_im", mo)

                    p_x3 = psum.tile([128, 2 * NB], F32, name=f"px{mo}",
                                     tag="bkC")
                    p_g3 = psum.tile([128, 2 * NB], F32, name=f"pg{mo}",
                                     tag="bkD")

                    def cand_accum(dst, g, mo, j0):
                        wrow = cand_idx[g] * 2 + mo
                        for k in range(4):
                            nc.tensor.matmul(
                                dst, w16t[:, wrow, k * 128:(k + 1) * 128],
                                m16[:, j0 + k, :], start=(k == 0),
                                stop=(k == 3))

                    cand_accum(p_x3[:, 0:NB], "x3_re", mo, 0)
                    cand_accum(p_x3[:, NB:], "x3_im", mo, 0)
                    cand_accum(p_g3[:, 0:NB], "g3_re", mo, 4)
                    cand_accum(p_g3[:, NB:], "g3_im", mo, 4)

                    # ---- elementwise epilogue ------------------------------
                    sr = spool.tile([128, 2 * NB], F16, name=f"sr{mo}",
                                    tag="sr")
                    sz = spool.tile([128, 2 * NB], F16, name=f"sz{mo}",
                                    tag="sz")
                    g3 = spool.tile([128, 2 * NB], F16, name=f"g3{mo}",
                                    tag="g3")
                    nc.scalar.activation(sr[:, 0:NB], p_r[:, 0:NB],
                                         AF.Sigmoid, bias=bias_ap("r_re", mo),
                                         scale=INV_S)
                    nc.scalar.activation(sr[:, NB:], p_r[:, NB:],
                                         AF.Sigmoid, bias=bias_ap("r_im", mo),
                                         scale=INV_S)
                    nc.scalar.activation(sz[:, 0:NB], p_z[:, 0:NB],
                                         AF.Sigmoid, bias=bias_ap("z_re", mo))
                    nc.scalar.activation(sz[:, NB:], p_z[:, NB:],
                                         AF.Sigmoid, bias=bias_ap("z_im", mo))
                    nc.scalar.activation(g3[:, 0:NB], p_g3[:, 0:NB],
                                         AF.Identity,
                                         bias=bias_ap("g3_re", mo))
                    nc.scalar.activation(g3[:, NB:], p_g3[:, NB:],
                                         AF.Identity,
                                         bias=bias_ap("g3_im", mo))

                    # h3 = r * g3 (complex), all fp16 SBUF (2x DVE mode)
                    u = tpool.tile([128, 2 * NB], F16, name=f"u{mo}", tag="u")
                    v = tpool.tile([128, 2 * NB], F16, name=f"v{mo}", tag="v")
                    h3 = tpool.tile([128, 2 * NB], F16, name=f"h3{mo}",
                                    tag="h3")
                    nc.vector.tensor_mul(u[:], sr[:], g3[:])
                    nc.vector.tensor_mul(v[:, 0:NB], sr[:, 0:NB], g3[:, NB:])
                    nc.vector.tensor_mul(v[:, NB:], sr[:, NB:], g3[:, 0:NB])
                    nc.vector.tensor_sub(h3[:, 0:NB], u[:, 0:NB], u[:, NB:])
                    nc.vector.tensor_add(h3[:, NB:], v[:, 0:NB], v[:, NB:])
                    # ss = x3 + h3 (PSUM read, 1x); tanh adds x3 bias
                    ss = tpool.tile([128, 2 * NB], F16, name=f"ss{mo}",
                                    tag="ss")
                    nc.vector.tensor_add(ss[:], p_x3[:], h3[:])
                    nn = spool.tile([128, 2 * NB], F16, name=f"nn{mo}",
                                    tag="nn")
                    nc.scalar.activation(nn[:, 0:NB], ss[:, 0:NB], AF.Tanh,
                                         bias=bias_ap("x3_re", mo))
                    nc.scalar.activation(nn[:, NB:], ss[:, NB:], AF.Tanh,
                                         bias=bias_ap("x3_im", mo))

                    # d = h - n ; out = n + z*d (complex)
                    d = tpool.tile([128, 2 * NB], F16, name=f"d{mo}", tag="d")
                    p = tpool.tile([128, 2 * NB], F16, name=f"p{mo}", tag="p")
                    q = tpool.tile([128, 2 * NB], F16, name=f"q{mo}", tag="q")
                    tm = tpool.tile([128, 2 * NB], F16, name=f"tm{mo}",
                                    tag="tm")
                    ot = opool.tile([128, 2, NB], F16, name=f"ot{mo}",
                                    tag="ot")
                    nc.vector.tensor_sub(d[:, 0:NB], m16[:, 4 + mo, :],
                                         nn[:, 0:NB])
                    nc.vector.tensor_sub(d[:, NB:], m16[:, 6 + mo, :],
                                         nn[:, NB:])
                    nc.vector.tensor_mul(p[:], sz[:], d[:])
                    nc.vector.tensor_mul(q[:, 0:NB], sz[:, 0:NB], d[:, NB:])
                    nc.vector.tensor_mul(q[:, NB:], sz[:, NB:], d[:, 0:NB])
                    nc.vector.tensor_sub(tm[:, 0:NB], p[:, 0:NB], p[:, NB:])
                    nc.vector.tensor_add(tm[:, NB:], q[:, 0:NB], q[:, NB:])
                    nc.vector.tensor_add(ot[:, 0, :], nn[:, 0:NB],
                                         tm[:, 0:NB])
                    nc.vector.tensor_add(ot[:, 1, :], nn[:, NB:], tm[:, NB:])

                    # one DMA per mo: [128 feat, 2 (re/im), NB]
                    nc.sync.dma_start(
                        outT[mo * 128:(mo + 1) * 128, :, c0:c0 + NB], ot[:])

    nc.compile()
    return nc


def _stack_stat(p, g):
    """Stationary matrix [K, 256] for accumulator g (K = 1024 or 512)."""
    blocks = {
        "r_re": [p["w1Wr"], -p["w1Wi"], p["r1Wr"], -p["r1Wi"]],
        "r_im": [p["w1Wi"], p["w1Wr"], p["r1Wi"], p["r1Wr"]],
        "z_re": [p["w2Wr"], -p["w2Wi"], p["r2Wr"], -p["r2Wi"]],
        "z_im": [p["w2Wi"], p["w2Wr"], p["r2Wi"], p["r2Wr"]],
        "x3_re": [p["w3Wr"], -p["w3Wi"]],
        "x3_im": [p["w3Wi"], p["w3Wr"]],
        "g3_re": [p["r3Wr"], -p["r3Wi"]],
        "g3_im": [p["r3Wi"], p["r3Wr"]],
    }[g]
    return np.concatenate([np.asarray(W, np.float32).T for W in blocks],
                          axis=0)


def _pack_gate_pair(w0, w1):
    """Pack a chunk pair [128,128]x2 into the DR weight layout [128, 256]."""
    if GATE_PERF == PM.DoubleRowSwInterleave:
        # flat[p, 2*(127-m) + i] = w_i[p, m]
        arr = np.stack([w0[:, ::-1], w1[:, ::-1]], axis=2)  # [p, m', i]
        return arr.reshape(128, 256)
    # plain DoubleRow: [p, i, m]
    return np.stack([w0, w1], axis=1).reshape(128, 256)


def _prep_weights(p):
    w8 = np.zeros((128, 4, 4 * 256), dtype=np.float32)
    for g in _R_ACCS:
        stat = _stack_stat(p, g)  # [1024, 256]
        for mo in range(2):
            sub = stat[:, mo * 128:(mo + 1) * 128] * S_WGT  # [1024, 128]
            gi = _R_ACCS.index(g)
            for si in range(4):
                w0 = sub[si * 256:si * 256 + 128]
                w1 = sub[si * 256 + 128:(si + 1) * 256]
                w8[:, gi * 2 + mo, si * 256:(si + 1) * 256] = \
                    _pack_gate_pair(w0, w1)
    w16z = np.zeros((128, 4, 8 * 128), dtype=np.float16)
    for g in _Z_ACCS:
        stat = _stack_stat(p, g)  # [1024, 256]
        for mo in range(2):
            sub = stat[:, mo * 128:(mo + 1) * 128]
            gi = _Z_ACCS.index(g)
            for k in range(8):
                w16z[:, gi * 2 + mo, k * 128:(k + 1) * 128] = \
                    sub[k * 128:(k + 1) * 128].astype(np.float16)
    w16 = np.zeros((128, 8, 4 * 128), dtype=np.float16)
    for g in _CAND_ACCS:
        stat = _stack_stat(p, g)  # [512, 256]
        for mo in range(2):
            sub = stat[:, mo * 128:(mo + 1) * 128]
            gi = _CAND_ACCS.index(g)
            for k in range(4):
                w16[:, gi * 2 + mo, k * 128:(k + 1) * 128] = \
                    sub[k * 128:(k + 1) * 128].astype(np.float16)

    bias = {
        "r_re": p["w1br"] - p["w1bi"] + p["r1br"] - p["r1bi"],
        "r_im": p["w1br"] + p["w1bi"] + p["r1br"] + p["r1bi"],
        "z_re": p["w2br"] - p["w2bi"] + p["r2br"] - p["r2bi"],
        "z_im": p["w2br"] + p["w2bi"] + p["r2br"] + p["r2bi"],
        "x3_re": p["w3br"] - p["w3bi"],
        "x3_im": p["w3br"] + p["w3bi"],
        "g3_re": p["r3br"] - p["r3bi"],
        "g3_im": p["r3br"] + p["r3bi"],
    }
    bcols = np.zeros((128, 16), dtype=np.float32)
    for gi, g in enumerate(_GATE_ACCS + _CAND_ACCS):
        for mo in range(2):
            bcols[:, gi * 2 + mo] = np.asarray(bias[g])[mo * 128:(mo + 1) * 128]
    return {
        "w8": np.clip(w8, -240.0, 240.0).astype(E4M3),
        "w16z": w16z,
        "w16": w16,
        "biases": bcols,
    }


def kernel(**inputs):
    global _CACHED_NC, LAST_RESULT
    if _CACHED_NC is None:
        _CACHED_NC = _build_nc()
    nc = _CACHED_NC

    wmaps = _prep_weights(inputs)

    in_maps = []
    for core in range(N_CORES):
        sl = slice(core * B_LOC, (core + 1) * B_LOC)
        m = dict(wmaps)
        # [4 streams][256 feat, 8192 batch]
        xT = np.stack([
            np.asarray(inputs[s], np.float32)[sl].T for s in _STREAMS])
        # -> [128, t, 2*si + c, b] : chunks c over feature dim
        xTi = (xT.reshape(4, 2, 128, N_TILES, NB)
               .transpose(2, 3, 0, 1, 4))          # [128, t, si, c, b]
        m["s16"] = np.ascontiguousarray(
            xTi.reshape(128, -1).astype(np.float16))
        sc = np.clip(xTi * S_MOV, -240.0, 240.0)
        m["s8h"] = np.ascontiguousarray(sc.astype(E4M3).reshape(128, -1))
        in_maps.append(m)

    kwargs = {}
    if TRACE:
        import sys, types
        try:
            from trn_agent_boot.trn_boot import _ntff_profile_via_ctypes
            mod = types.ModuleType("antenv.axon_hooks")
            mod._hook = _ntff_profile_via_ctypes('/opt/axon/libaxon_pjrt.so')
            mod.get_axon_ntff_profile_hook = lambda: mod._hook
            mod.set_axon_ntff_profile_hook = (
                lambda h: setattr(mod, "_hook", h))
            sys.modules["antenv.axon_hooks"] = mod
            kwargs["trace"] = True
        except Exception:
            pass

    res = bass_utils.run_bass_kernel_spmd(
        nc, in_maps, core_ids=list(range(N_CORES)), **kwargs)
    LAST_RESULT = res

    out = np.empty((2, B_TOTAL, H), dtype=np.float32)
    for core in range(N_CORES):
        sl = slice(core * B_LOC, (core + 1) * B_LOC)
        o = np.asarray(res.results[core]["outT"], dtype=np.float32)
        out[0, sl] = o[:, 0, :].T
        out[1, sl] = o[:, 1, :].T
    return out
